# revision 25
# baseline (speedup 1.0000x reference)
"""CenterLoss kernel for Trainium2 (Bass/Tile), data-parallel over 8 NeuronCores.

loss = 0.5 * sum_i ||x_i - centers[targets_i]||^2

The reference materializes the full [N, C] distance matrix and gathers one
entry per row; here we gather only the 512 target center rows per core and
do a fused add / square-accumulate.

Sharding: inputs/targets split along batch N across 8 cores (512 rows each),
centers replicated. Each core PE-reduces its per-partition partials to a
[1, 8] row and ships 32 bytes; the host sums across cores and scales by 0.5.

Design notes (all measured on HW traces; traced time 34.8 us -> 23.3 us):
  - The gather uses gpsimd `indirect_dma_start` (the resident SWDGE
    IndirectCopy path, one row per partition per 128-row chunk) rather than
    `dma_gather`: dma_gather lives in the loadable `mlp` ucode library whose
    ~11 us IRAM load gates the first descriptor. Exactly [128, 1]-shaped
    offsets per op — multi-offset-per-partition ([128, 2]) and CCE
    compute_op variants pass CoreSim but crash the HW DGE.
  - Both x and centers live in HBM as fp8 e4m3 (1.5 MB/core total HBM
    traffic); the SWDGE DMAs upcast to bf16 in flight so SBUF compute keeps
    the DVE 2x mode (cayman DVE has no fp8 packing). Loss rel-err ~9e-4
    vs the 2e-2 budget.
  - idx rides the SP HWDGE ring: SDMA queue rows drain in priority order,
    so the 2 KB idx transfer must not share a ring with (or sit behind) the
    x stream — idx-ready gates the first gather descriptor-gen.
  - x arrives via two SWDGE cast-DMAs emitted by the Q7 at body start,
    before the gather descgens queue up; their descriptors drain ahead of
    the gathers on the same ring, so x chunks 0-1 are in SBUF before the
    first gather lands.
  - Per chunk: DVE adds -x (host ships x negated), then the square+row-sum
    is split FA/(1024-FA) between ACT (fused square+accumulate, one
    READ_ACCUMULATOR per chunk) and DVE (mult+reduce) so neither engine is
    the serial tail; the last chunk leans harder on DVE.
  - The final partition-reduce runs on the otherwise-idle PE (ones^T @ acc)
    so the output DMA is one 32-byte descriptor; a [128, 8] store costs
    ~2 us more in small-descriptor drain and write receipts.
"""

import numpy as np
import ml_dtypes

import concourse.bacc as bacc
import concourse.bass as bass
import concourse.tile as tile
from concourse import mybir
from concourse.bass_utils import run_bass_kernel_spmd

N, C, D = 4096, 8192, 1024
N_CORES = 8
ROWS = N // N_CORES  # 512 rows per core
P = 128              # SBUF partitions
CHUNKS = ROWS // P   # 4 chunks of 128 rows
NACC = 2 * CHUNKS    # per chunk: one ACT accum col + one DVE reduce col
FA = 832             # cols squared on ACT per chunk (rest: DVE mult+reduce)
FA_LAST = 512        # last chunk leans harder on DVE to shorten the tail

BF16 = mybir.dt.bfloat16

# Stashed BassKernelResults from the most recent kernel() call (for profiling).
LAST_RESULTS = None
_NC_CACHE = None


def _build_bass():
    nc = bacc.Bacc("TRN2", target_bir_lowering=False)
    x = nc.dram_tensor("x", [P, CHUNKS * D], mybir.dt.float8e4, kind="ExternalInput")
    idx = nc.dram_tensor("idx", [P, CHUNKS], mybir.dt.int32, kind="ExternalInput")
    centers = nc.dram_tensor("centers", [C, D], mybir.dt.float8e4, kind="ExternalInput")
    out = nc.dram_tensor("out", [1, NACC], mybir.dt.float32, kind="ExternalOutput")

    with tile.TileContext(nc) as tc:
        with (
            tc.tile_pool(name="io", bufs=1) as io,
            tc.tile_pool(name="cpool", bufs=CHUNKS) as cp,
            tc.tile_pool(name="psum", bufs=1, space="PSUM") as pp,
            tc.tile_pool(name="small", bufs=1) as small,
        ):
            # idx first on the SP ring (HWDGE; measured ~2.1 us issue->sem,
            # faster than the pool SWDGE route whose Q7 emission queues
            # behind framework preamble work).
            idx_sb = small.tile([P, CHUNKS], mybir.dt.int32)
            nc.sync.dma_start(idx_sb[:], idx[:, :])

            ones = small.tile([P, 1], mybir.dt.float32)
            nc.vector.memset(ones[:], 1.0)
            # Dummy activation to pull the ACT function-table load off the
            # critical path.
            warm = small.tile([1, 1], mybir.dt.float32)
            nc.scalar.activation(
                out=warm[:], in_=ones[0:1, :],
                func=mybir.ActivationFunctionType.Square,
            )

            # -x, fp8, pre-swizzled: column block t of partition p holds
            # shard row t*128 + p (matching the gather's one-row-per-
            # partition output). Two SWDGE cast-DMAs (fp8 -> bf16 in
            # flight), emitted while the pool queue is otherwise idle; their
            # descriptors drain ahead of the gathers' on the same ring.
            x_sb = io.tile([P, CHUNKS * D], BF16, tag="x")
            half = (CHUNKS // 2) * D
            nc.gpsimd.dma_start(x_sb[:, 0:half], x[:, 0:half])
            nc.gpsimd.dma_start(x_sb[:, half:], x[:, half:])

            acc = small.tile([P, NACC], mybir.dt.float32)
            # Four 128-row gathers (one row per partition per op).
            ct = []
            for t in range(CHUNKS):
                cg = cp.tile([P, D], BF16, tag=f"c{t}")
                nc.gpsimd.indirect_dma_start(
                    out=cg[:],
                    out_offset=None,
                    in_=centers[:, :],
                    in_offset=bass.IndirectOffsetOnAxis(
                        ap=idx_sb[:, t : t + 1], axis=0
                    ),
                )
                ct.append(cg)
            for t in range(CHUNKS):
                fa = FA_LAST if t == CHUNKS - 1 else FA
                dv = ct[t][:]
                xv = x_sb[:, t * D : (t + 1) * D]
                # d = c + (-x)
                nc.vector.tensor_add(dv, dv, xv)
                # acc col 2t = sum_{d<fa} d^2 (ACT fused square+row-sum)
                nc.scalar.activation(
                    out=dv[:, 0:fa],
                    in_=dv[:, 0:fa],
                    func=mybir.ActivationFunctionType.Square,
                    accum_out=acc[:, 2 * t : 2 * t + 1],
                )
                # acc col 2t+1 = sum_{d>=fa} d^2 (DVE mult+reduce)
                h1 = dv[:, fa:D]
                nc.vector.tensor_tensor(h1, h1, h1, op=mybir.AluOpType.mult)
                nc.vector.tensor_reduce(
                    acc[:, 2 * t + 1 : 2 * t + 2],
                    h1,
                    axis=mybir.AxisListType.X,
                    op=mybir.AluOpType.add,
                )
            # Partition-reduce on the (otherwise idle) PE: ones^T @ acc gives
            # [1, NACC]; one 32-byte, single-descriptor DMA ships it. (A
            # [128, NACC] store costs ~2 us more in small-descriptor drain
            # and write receipts.)
            psum = pp.tile([1, NACC], mybir.dt.float32, tag="ps")
            nc.tensor.matmul(
                psum[:], lhsT=ones[:], rhs=acc[:, :], start=True, stop=True
            )
            res = small.tile([1, NACC], mybir.dt.float32)
            nc.vector.tensor_copy(res[:], psum[:])
            nc.sync.dma_start(out[:, :], res[:])
    nc.finalize()
    return nc


def _get_nc():
    global _NC_CACHE
    if _NC_CACHE is None:
        _NC_CACHE = _build_bass()
    return _NC_CACHE


def kernel(inputs, targets, centers):
    global LAST_RESULTS
    x = np.asarray(inputs, dtype=np.float32)
    tgt = np.asarray(targets).astype(np.int32)
    cen = np.ascontiguousarray(
        np.asarray(centers, dtype=np.float32).astype(ml_dtypes.float8_e4m3)
    )
    assert x.shape == (N, D) and cen.shape == (C, D) and tgt.shape == (N,)

    xneg = (-x).astype(ml_dtypes.float8_e4m3)
    nc = _get_nc()
    in_maps = []
    for c in range(N_CORES):
        xs = xneg[c * ROWS : (c + 1) * ROWS]
        # partition p, column block t <- shard row t*128 + p
        xw = np.ascontiguousarray(
            xs.reshape(CHUNKS, P, D).transpose(1, 0, 2).reshape(P, CHUNKS * D)
        )
        # idx[p, t] = target row for shard row t*128 + p
        ts = tgt[c * ROWS : (c + 1) * ROWS]
        idxw = np.ascontiguousarray(ts.reshape(CHUNKS, P).T)
        in_maps.append({"x": xw, "idx": idxw, "centers": cen})

    res = run_bass_kernel_spmd(nc, in_maps, core_ids=list(range(N_CORES)))
    LAST_RESULTS = res

    total = 0.0
    for r in res.results:
        total += float(r["out"].astype(np.float64).sum())
    return np.array(0.5 * total, dtype=np.float32)


# revision 27
# speedup vs baseline: 1.0089x; 1.0089x over previous
"""CenterLoss kernel for Trainium2 (Bass/Tile), data-parallel over 8 NeuronCores.

loss = 0.5 * sum_i ||x_i - centers[targets_i]||^2

The reference materializes the full [N, C] distance matrix and gathers one
entry per row; here we gather only the 512 target center rows per core and
do a fused add / square-accumulate.

Sharding: inputs/targets split along batch N across 8 cores (512 rows each),
centers replicated. Each core PE-reduces its per-partition partials to a
[1, 8] row and ships 32 bytes; the host sums across cores and scales by 0.5.

Design notes (all measured on HW traces; traced time 34.8 us -> 23.3 us):
  - The gather uses gpsimd `indirect_dma_start` (the resident SWDGE
    IndirectCopy path, one row per partition per 128-row chunk) rather than
    `dma_gather`: dma_gather lives in the loadable `mlp` ucode library whose
    ~11 us IRAM load gates the first descriptor. Exactly [128, 1]-shaped
    offsets per op — multi-offset-per-partition ([128, 2]) and CCE
    compute_op variants pass CoreSim but crash the HW DGE.
  - Both x and centers live in HBM as fp8 e4m3 (1.5 MB/core total HBM
    traffic); the SWDGE DMAs upcast to bf16 in flight so SBUF compute keeps
    the DVE 2x mode (cayman DVE has no fp8 packing). Loss rel-err ~9e-4
    vs the 2e-2 budget.
  - idx rides the SP HWDGE ring: SDMA queue rows drain in priority order,
    so the 2 KB idx transfer must not share a ring with (or sit behind) the
    x stream — idx-ready gates the first gather descriptor-gen.
  - x arrives via two SWDGE cast-DMAs emitted by the Q7 at body start,
    before the gather descgens queue up; their descriptors drain ahead of
    the gathers on the same ring, so x chunks 0-1 are in SBUF before the
    first gather lands.
  - Per chunk: DVE adds -x (host ships x negated), then the square+row-sum
    is split FA/(1024-FA) between ACT (fused square+accumulate, one
    READ_ACCUMULATOR per chunk) and DVE (mult+reduce) so neither engine is
    the serial tail; the last chunk leans harder on DVE.
  - The final partition-reduce runs on the otherwise-idle PE (ones^T @ acc)
    so the output DMA is one 32-byte descriptor; a [128, 8] store costs
    ~2 us more in small-descriptor drain and write receipts.
"""

import numpy as np
import ml_dtypes

import concourse.bacc as bacc
import concourse.bass as bass
import concourse.tile as tile
from concourse import mybir
from concourse.bass_utils import run_bass_kernel_spmd

N, C, D = 4096, 8192, 1024
N_CORES = 8
ROWS = N // N_CORES  # 512 rows per core
P = 128              # SBUF partitions
CHUNKS = ROWS // P   # 4 chunks of 128 rows
WB = 192             # DVE-side cols for chunks 0..2 (1024 - FA)
WL = 512             # DVE-side cols for the last chunk (1024 - FA_LAST)
NOUT = WB + WL + CHUNKS  # DVE col-sums (PE-reduced) + ACT accum cols
FA = 832             # cols squared on ACT per chunk (rest: DVE mult+reduce)
FA_LAST = 512        # last chunk leans harder on DVE to shorten the tail

BF16 = mybir.dt.bfloat16

# Stashed BassKernelResults from the most recent kernel() call (for profiling).
LAST_RESULTS = None
_NC_CACHE = None


def _build_bass():
    nc = bacc.Bacc("TRN2", target_bir_lowering=False)
    x = nc.dram_tensor("x", [P, CHUNKS * D], mybir.dt.float8e4, kind="ExternalInput")
    idx = nc.dram_tensor("idx", [P, CHUNKS], mybir.dt.int32, kind="ExternalInput")
    centers = nc.dram_tensor("centers", [C, D], mybir.dt.float8e4, kind="ExternalInput")
    out = nc.dram_tensor("out", [1, NOUT], mybir.dt.float32, kind="ExternalOutput")

    with tile.TileContext(nc) as tc:
        with (
            tc.tile_pool(name="io", bufs=1) as io,
            tc.tile_pool(name="cpool", bufs=CHUNKS) as cp,
            tc.tile_pool(name="psum", bufs=1, space="PSUM") as pp,
            tc.tile_pool(name="small", bufs=1) as small,
        ):
            # idx first on the SP ring (HWDGE; measured ~2.1 us issue->sem,
            # faster than the pool SWDGE route whose Q7 emission queues
            # behind framework preamble work).
            idx_sb = small.tile([P, CHUNKS], mybir.dt.int32)
            nc.sync.dma_start(idx_sb[:, 0:1], idx[:, 0:1])
            nc.sync.dma_start(idx_sb[:, 1:], idx[:, 1:])

            ones = small.tile([P, 1], mybir.dt.float32)
            nc.vector.memset(ones[:], 1.0)
            ones_bf = small.tile([P, 1], BF16)
            nc.vector.memset(ones_bf[:], 1.0)
            # Dummy activation to pull the ACT function-table load off the
            # critical path.
            warm = small.tile([1, 1], mybir.dt.float32)
            nc.scalar.activation(
                out=warm[:], in_=ones[0:1, :],
                func=mybir.ActivationFunctionType.Square,
            )

            # -x, fp8, pre-swizzled: column block t of partition p holds
            # shard row t*128 + p (matching the gather's one-row-per-
            # partition output). Two SWDGE cast-DMAs (fp8 -> bf16 in
            # flight), emitted while the pool queue is otherwise idle; their
            # descriptors drain ahead of the gathers' on the same ring.
            x_sb = io.tile([P, CHUNKS * D], BF16, tag="x")
            half = (CHUNKS // 2) * D
            nc.gpsimd.dma_start(x_sb[:, 0:half], x[:, 0:half])
            nc.gpsimd.dma_start(x_sb[:, half:], x[:, half:])

            acc = small.tile([P, CHUNKS], mybir.dt.float32)
            psum_a = pp.tile([1, WB], mybir.dt.float32, tag="pa")
            psum_b = pp.tile([1, WL], mybir.dt.float32, tag="pb")
            # Four 128-row gathers (one row per partition per op).
            ct = []
            for t in range(CHUNKS):
                cg = cp.tile([P, D], BF16, tag=f"c{t}")
                nc.gpsimd.indirect_dma_start(
                    out=cg[:],
                    out_offset=None,
                    in_=centers[:, :],
                    in_offset=bass.IndirectOffsetOnAxis(
                        ap=idx_sb[:, t : t + 1], axis=0
                    ),
                )
                ct.append(cg)
            for t in range(CHUNKS):
                fa = FA_LAST if t == CHUNKS - 1 else FA
                dv = ct[t][:]
                xv = x_sb[:, t * D : (t + 1) * D]
                # d = c + (-x)
                nc.vector.tensor_add(dv, dv, xv)
                # acc col t = sum_{d<fa} d^2 (ACT fused square+row-sum)
                nc.scalar.activation(
                    out=dv[:, 0:fa],
                    in_=dv[:, 0:fa],
                    func=mybir.ActivationFunctionType.Square,
                    accum_out=acc[:, t : t + 1],
                )
                # Tail cols: DVE squares in place; the (otherwise idle) PE
                # column-sums them via ones^T, accumulating chunks 0..2 in
                # PSUM. The host sums the resulting columns — this replaces
                # the DVE tensor_reduce (~0.3-0.7 us per chunk).
                h1 = dv[:, fa:D]
                nc.vector.tensor_tensor(h1, h1, h1, op=mybir.AluOpType.mult)
                if t < CHUNKS - 1:
                    nc.tensor.matmul(
                        psum_a[:], lhsT=ones_bf[:], rhs=h1,
                        start=(t == 0), stop=(t == CHUNKS - 2),
                    )
                else:
                    nc.tensor.matmul(
                        psum_b[:], lhsT=ones_bf[:], rhs=h1, start=True, stop=True
                    )
            # Partition-reduce on the (otherwise idle) PE: ones^T @ acc gives
            # [1, NACC]; one 32-byte, single-descriptor DMA ships it. (A
            # [128, NACC] store costs ~2 us more in small-descriptor drain
            # and write receipts.)
            psum_c = pp.tile([1, CHUNKS], mybir.dt.float32, tag="pc")
            nc.tensor.matmul(
                psum_c[:], lhsT=ones[:], rhs=acc[:, :], start=True, stop=True
            )
            res = small.tile([1, NOUT], mybir.dt.float32)
            nc.vector.tensor_copy(res[:, 0:WB], psum_a[:])
            nc.vector.tensor_copy(res[:, WB : WB + WL], psum_b[:])
            nc.vector.tensor_copy(res[:, WB + WL :], psum_c[:])
            nc.sync.dma_start(out[:, :], res[:])
    nc.finalize()
    return nc


def _get_nc():
    global _NC_CACHE
    if _NC_CACHE is None:
        _NC_CACHE = _build_bass()
    return _NC_CACHE


def kernel(inputs, targets, centers):
    global LAST_RESULTS
    x = np.asarray(inputs, dtype=np.float32)
    tgt = np.asarray(targets).astype(np.int32)
    cen = np.ascontiguousarray(
        np.asarray(centers, dtype=np.float32).astype(ml_dtypes.float8_e4m3)
    )
    assert x.shape == (N, D) and cen.shape == (C, D) and tgt.shape == (N,)

    xneg = (-x).astype(ml_dtypes.float8_e4m3)
    nc = _get_nc()
    in_maps = []
    for c in range(N_CORES):
        xs = xneg[c * ROWS : (c + 1) * ROWS]
        # partition p, column block t <- shard row t*128 + p
        xw = np.ascontiguousarray(
            xs.reshape(CHUNKS, P, D).transpose(1, 0, 2).reshape(P, CHUNKS * D)
        )
        # idx[p, t] = target row for shard row t*128 + p
        ts = tgt[c * ROWS : (c + 1) * ROWS]
        idxw = np.ascontiguousarray(ts.reshape(CHUNKS, P).T)
        in_maps.append({"x": xw, "idx": idxw, "centers": cen})

    res = run_bass_kernel_spmd(nc, in_maps, core_ids=list(range(N_CORES)))
    LAST_RESULTS = res

    total = 0.0
    for r in res.results:
        total += float(r["out"].astype(np.float64).sum())
    return np.array(0.5 * total, dtype=np.float32)


# revision 29
# speedup vs baseline: 1.0252x; 1.0162x over previous
"""CenterLoss kernel for Trainium2 (Bass/Tile), data-parallel over 8 NeuronCores.

loss = 0.5 * sum_i ||x_i - centers[targets_i]||^2

The reference materializes the full [N, C] distance matrix and gathers one
entry per row; here we gather only the 512 target center rows per core and
do a fused add / square-accumulate.

Sharding: inputs/targets split along batch N across 8 cores (512 rows each),
centers replicated. Each core PE-reduces its partials to a [1, 708] row
(DVE-side column sums + ACT accumulator columns) shipped in one
single-descriptor DMA; the host sums across cores and scales by 0.5.

Design notes (all measured on HW traces; traced time 34.8 us -> 23.3 us):
  - The gather uses gpsimd `indirect_dma_start` (the resident SWDGE
    IndirectCopy path, one row per partition per 128-row chunk) rather than
    `dma_gather`: dma_gather lives in the loadable `mlp` ucode library whose
    ~11 us IRAM load gates the first descriptor. Exactly [128, 1]-shaped
    offsets per op — multi-offset-per-partition ([128, 2]) and CCE
    compute_op variants pass CoreSim but crash the HW DGE.
  - Both x and centers live in HBM as fp8 e4m3 (1.5 MB/core total HBM
    traffic); the SWDGE DMAs upcast to bf16 in flight so SBUF compute keeps
    the DVE 2x mode (cayman DVE has no fp8 packing). Loss rel-err ~9e-4
    vs the 2e-2 budget.
  - idx rides the SP HWDGE ring: SDMA queue rows drain in priority order,
    so the 2 KB idx transfer must not share a ring with (or sit behind) the
    x stream — idx-ready gates the first gather descriptor-gen.
  - x arrives via two SWDGE cast-DMAs emitted by the Q7 at body start,
    before the gather descgens queue up; their descriptors drain ahead of
    the gathers on the same ring, so x chunks 0-1 are in SBUF before the
    first gather lands.
  - Per chunk: DVE adds -x (host ships x negated), then the square+row-sum
    is split FA/(1024-FA) between ACT (fused square+accumulate, one
    READ_ACCUMULATOR per chunk) and DVE (in-place square) so neither engine
    is the serial tail; the last chunk leans harder on DVE.
  - All reductions land on the otherwise-idle PE: ones^T column-sums the
    DVE-side squares (PSUM-accumulated across chunks, replacing a DVE
    tensor_reduce per chunk) and partition-reduces the ACT accumulator
    columns, so the output DMA is one descriptor; a [128, k] store costs
    ~2 us more in small-descriptor drain and write receipts. (DVE
    tensor_tensor_reduce would fuse square+row-sum but crashes on HW.)
"""

import numpy as np
import ml_dtypes

import concourse.bacc as bacc
import concourse.bass as bass
import concourse.tile as tile
from concourse import mybir
from concourse.bass_utils import run_bass_kernel_spmd

N, C, D = 4096, 8192, 1024
N_CORES = 8
ROWS = N // N_CORES  # 512 rows per core
P = 128              # SBUF partitions
CHUNKS = ROWS // P   # 4 chunks of 128 rows
WB = 256             # DVE-side cols for chunks 0..2 (1024 - FA)
WL = 512             # DVE-side cols for the last chunk (1024 - FA_LAST)
NOUT = WB + WL + CHUNKS  # DVE col-sums (PE-reduced) + ACT accum cols
FA = 768             # cols squared on ACT per chunk (rest: DVE square)
FA_LAST = 512        # last chunk leans harder on DVE to shorten the tail

BF16 = mybir.dt.bfloat16

# Stashed BassKernelResults from the most recent kernel() call (for profiling).
LAST_RESULTS = None
_NC_CACHE = None


def _build_bass():
    nc = bacc.Bacc("TRN2", target_bir_lowering=False)
    x = nc.dram_tensor("x", [P, CHUNKS * D], mybir.dt.float8e4, kind="ExternalInput")
    idx = nc.dram_tensor("idx", [P, CHUNKS], mybir.dt.int32, kind="ExternalInput")
    centers = nc.dram_tensor("centers", [C, D], mybir.dt.float8e4, kind="ExternalInput")
    out = nc.dram_tensor("out", [1, NOUT], mybir.dt.float32, kind="ExternalOutput")

    with tile.TileContext(nc) as tc:
        with (
            tc.tile_pool(name="io", bufs=1) as io,
            tc.tile_pool(name="cpool", bufs=CHUNKS) as cp,
            tc.tile_pool(name="psum", bufs=1, space="PSUM") as pp,
            tc.tile_pool(name="small", bufs=1) as small,
        ):
            # idx first on the SP ring (HWDGE; measured ~2.1 us issue->sem,
            # faster than the pool SWDGE route whose Q7 emission queues
            # behind framework preamble work).
            idx_sb = small.tile([P, CHUNKS], mybir.dt.int32)
            nc.sync.dma_start(idx_sb[:, 0:1], idx[:, 0:1])
            nc.sync.dma_start(idx_sb[:, 1:], idx[:, 1:])

            ones = small.tile([P, 1], mybir.dt.float32)
            nc.vector.memset(ones[:], 1.0)
            ones_bf = small.tile([P, 1], BF16)
            nc.vector.memset(ones_bf[:], 1.0)
            # Dummy activation to pull the ACT function-table load off the
            # critical path.
            warm = small.tile([1, 1], mybir.dt.float32)
            nc.scalar.activation(
                out=warm[:], in_=ones[0:1, :],
                func=mybir.ActivationFunctionType.Square,
            )

            # -x, fp8, pre-swizzled: column block t of partition p holds
            # shard row t*128 + p (matching the gather's one-row-per-
            # partition output). Two SWDGE cast-DMAs (fp8 -> bf16 in
            # flight), emitted while the pool queue is otherwise idle; their
            # descriptors drain ahead of the gathers' on the same ring.
            x_sb = io.tile([P, CHUNKS * D], BF16, tag="x")
            half = (CHUNKS // 2) * D
            nc.gpsimd.dma_start(x_sb[:, 0:half], x[:, 0:half])
            nc.gpsimd.dma_start(x_sb[:, half:], x[:, half:])

            acc = small.tile([P, CHUNKS], mybir.dt.float32)
            psum_a = pp.tile([1, WB], mybir.dt.float32, tag="pa")
            psum_b = pp.tile([1, WL], mybir.dt.float32, tag="pb")
            # Four 128-row gathers (one row per partition per op).
            ct = []
            for t in range(CHUNKS):
                cg = cp.tile([P, D], BF16, tag=f"c{t}")
                nc.gpsimd.indirect_dma_start(
                    out=cg[:],
                    out_offset=None,
                    in_=centers[:, :],
                    in_offset=bass.IndirectOffsetOnAxis(
                        ap=idx_sb[:, t : t + 1], axis=0
                    ),
                )
                ct.append(cg)
            for t in range(CHUNKS):
                fa = FA_LAST if t == CHUNKS - 1 else FA
                dv = ct[t][:]
                xv = x_sb[:, t * D : (t + 1) * D]
                # d = c + (-x)
                nc.vector.tensor_add(dv, dv, xv)
                # acc col t = sum_{d<fa} d^2 (ACT fused square+row-sum)
                nc.scalar.activation(
                    out=dv[:, 0:fa],
                    in_=dv[:, 0:fa],
                    func=mybir.ActivationFunctionType.Square,
                    accum_out=acc[:, t : t + 1],
                )
                # Tail cols: DVE squares in place; the (otherwise idle) PE
                # column-sums them via ones^T, accumulating chunks 0..2 in
                # PSUM. The host sums the resulting columns — this replaces
                # the DVE tensor_reduce (~0.3-0.7 us per chunk).
                h1 = dv[:, fa:D]
                nc.vector.tensor_tensor(h1, h1, h1, op=mybir.AluOpType.mult)
                if t < CHUNKS - 1:
                    nc.tensor.matmul(
                        psum_a[:], lhsT=ones_bf[:], rhs=h1,
                        start=(t == 0), stop=(t == CHUNKS - 2),
                    )
                else:
                    nc.tensor.matmul(
                        psum_b[:], lhsT=ones_bf[:], rhs=h1, start=True, stop=True
                    )
            # Partition-reduce on the (otherwise idle) PE: ones^T @ acc gives
            # [1, NACC]; one 32-byte, single-descriptor DMA ships it. (A
            # [128, NACC] store costs ~2 us more in small-descriptor drain
            # and write receipts.)
            psum_c = pp.tile([1, CHUNKS], mybir.dt.float32, tag="pc")
            nc.tensor.matmul(
                psum_c[:], lhsT=ones[:], rhs=acc[:, :], start=True, stop=True
            )
            res = small.tile([1, NOUT], mybir.dt.float32)
            nc.vector.tensor_copy(res[:, 0:WB], psum_a[:])
            nc.vector.tensor_copy(res[:, WB : WB + WL], psum_b[:])
            nc.vector.tensor_copy(res[:, WB + WL :], psum_c[:])
            nc.sync.dma_start(out[:, :], res[:])
    nc.finalize()
    return nc


def _get_nc():
    global _NC_CACHE
    if _NC_CACHE is None:
        _NC_CACHE = _build_bass()
    return _NC_CACHE


def kernel(inputs, targets, centers):
    global LAST_RESULTS
    x = np.asarray(inputs, dtype=np.float32)
    tgt = np.asarray(targets).astype(np.int32)
    cen = np.ascontiguousarray(
        np.asarray(centers, dtype=np.float32).astype(ml_dtypes.float8_e4m3)
    )
    assert x.shape == (N, D) and cen.shape == (C, D) and tgt.shape == (N,)

    xneg = (-x).astype(ml_dtypes.float8_e4m3)
    nc = _get_nc()
    in_maps = []
    for c in range(N_CORES):
        xs = xneg[c * ROWS : (c + 1) * ROWS]
        # partition p, column block t <- shard row t*128 + p
        xw = np.ascontiguousarray(
            xs.reshape(CHUNKS, P, D).transpose(1, 0, 2).reshape(P, CHUNKS * D)
        )
        # idx[p, t] = target row for shard row t*128 + p
        ts = tgt[c * ROWS : (c + 1) * ROWS]
        idxw = np.ascontiguousarray(ts.reshape(CHUNKS, P).T)
        in_maps.append({"x": xw, "idx": idxw, "centers": cen})

    res = run_bass_kernel_spmd(nc, in_maps, core_ids=list(range(N_CORES)))
    LAST_RESULTS = res

    total = 0.0
    for r in res.results:
        total += float(r["out"].astype(np.float64).sum())
    return np.array(0.5 * total, dtype=np.float32)
